# revision 17
# baseline (speedup 1.0000x reference)
"""Column-sum kernel for Trainium2: out[d] = sum_r x[r, d].

x is [8192, 4096] f32, rows sharded across 8 NeuronCores (1024 rows
each). Per core the shard is loaded as COLUMN blocks: one DMA brings
all 1024 rows of a W-column block into SBUF as 8 side-by-side
[128, W] sub-tiles. DVE folds the 8 sub-tiles into one [128, W]
accumulator (7 in-place adds, hidden under the next block's DMA), the
PE closes that block's ones-matmul partition reduce immediately, and
ACT copies PSUM out. Column blocks keep the reduce pipelined with the
load stream, so after the last DMA only one small block's fold +
matmul remains (the last block is deliberately narrow). fp32 PE
matmuls are half-rate (LOW_HIGH), so keeping PE work to one matmul
per 512 columns total — instead of one per tile — is what makes this
memory-bound instead of PE-bound. Host sums the 8 per-core partials.
"""

import numpy as np

M_CORES = 8
ROWS, D = 8192, 4096
ROWS_PER_CORE = ROWS // M_CORES  # 1024
P = 128
J_TILES = ROWS_PER_CORE // P  # 8 row sub-tiles per column block
BLOCK_W = (640, 640, 640, 640, 640, 640, 256)  # sums to 4096
NCHUNK = 512  # fp32 PSUM bank capacity / max fp32 moving free dim

_nc_cache = None


def _build():
    import concourse.tile as tile
    from concourse import bacc, mybir

    nc = bacc.Bacc(None)
    x = nc.declare_dram_parameter(
        "x", [ROWS_PER_CORE, D], mybir.dt.float32, isOutput=False
    )
    out = nc.declare_dram_parameter("out", [1, D], mybir.dt.float32, isOutput=True)

    x3 = x[:].rearrange("(j p) d -> p j d", p=P)  # [128, 8, 4096]

    with tile.TileContext(nc) as tc:
        with (
            tc.tile_pool(name="xpool", bufs=4) as xpool,
            tc.tile_pool(name="accpool", bufs=2) as accpool,
            tc.tile_pool(name="singles", bufs=1) as singles,
            tc.tile_pool(name="psum", bufs=4, space="PSUM") as psum_pool,
        ):
            ones = singles.tile([P, 1], mybir.dt.float32)
            nc.vector.memset(ones[:], 1.0)

            osb = singles.tile([1, D], mybir.dt.float32)

            col = 0
            for b, W in enumerate(BLOCK_W):
                xt = xpool.tile([P, J_TILES * W], mybir.dt.float32,
                                name=f"xt{b}", tag="xt")
                nc.sync.dma_start(
                    xt[:].rearrange("p (j w) -> p j w", j=J_TILES),
                    x3[:, :, col : col + W],
                )

                # Tree fold of the 8 sub-tiles, split across DVE and GpSimd
                # so neither engine saturates (GpSimd 2-input runs ~2x
                # slower, so it gets independent level-1 pairs only).
                def sub(j):
                    return xt[:, j * W : (j + 1) * W]

                t01 = accpool.tile([P, W], mybir.dt.float32, name=f"t01_{b}", tag="w0")
                t23 = accpool.tile([P, W], mybir.dt.float32, name=f"t23_{b}", tag="w1")
                t45 = accpool.tile([P, W], mybir.dt.float32, name=f"t45_{b}", tag="w2")
                t67 = accpool.tile([P, W], mybir.dt.float32, name=f"t67_{b}", tag="w3")
                nc.vector.tensor_add(t01[:], sub(0), sub(1))
                nc.gpsimd.tensor_add(t23[:], sub(2), sub(3))
                nc.vector.tensor_add(t45[:], sub(4), sub(5))
                nc.gpsimd.tensor_add(t67[:], sub(6), sub(7))
                h0 = accpool.tile([P, W], mybir.dt.float32, name=f"h0_{b}", tag="w4")
                h1 = accpool.tile([P, W], mybir.dt.float32, name=f"h1_{b}", tag="w5")
                nc.vector.tensor_add(h0[:], t01[:], t45[:])
                nc.vector.tensor_add(h1[:], t23[:], t67[:])
                acc = accpool.tile([P, W], mybir.dt.float32, name=f"acc{b}", tag="w6")
                nc.vector.tensor_add(acc[:], h0[:], h1[:])

                for s0 in range(0, W, NCHUNK):
                    sw = min(NCHUNK, W - s0)
                    ps = psum_pool.tile([1, NCHUNK], mybir.dt.float32,
                                        name=f"ps{b}_{s0}", tag="ps")
                    nc.tensor.matmul(
                        ps[:1, :sw], ones[:], acc[:, s0 : s0 + sw],
                        start=True, stop=True,
                    )
                    nc.scalar.copy(osb[:, col + s0 : col + s0 + sw], ps[:1, :sw])
                col += W

            nc.sync.dma_start(out[:, :], osb[:])

    nc.compile()
    return nc


def _get_nc():
    global _nc_cache
    if _nc_cache is None:
        _nc_cache = _build()
    return _nc_cache


def _run(x_np: np.ndarray, **run_kwargs):
    from concourse.bass_utils import run_bass_kernel_spmd

    nc = _get_nc()
    shards = np.split(x_np, M_CORES, axis=0)
    in_maps = [{"x": np.ascontiguousarray(s)} for s in shards]
    return run_bass_kernel_spmd(nc, in_maps, list(range(M_CORES)), **run_kwargs)


def kernel(x) -> np.ndarray:
    x_np = np.ascontiguousarray(np.asarray(x), dtype=np.float32)
    assert x_np.shape == (ROWS, D), x_np.shape
    res = _run(x_np)
    partials = np.stack([r["out"][0] for r in res.results])
    return partials.sum(axis=0, dtype=np.float32)


# revision 18
# speedup vs baseline: 1.1644x; 1.1644x over previous
"""Column-sum kernel for Trainium2: out[d] = sum_r x[r, d].

x is [8192, 4096] f32, rows sharded across 8 NeuronCores (1024 rows
each). Per-core pipeline:

- Rows 0..767 load as six contiguous [128, 4096] row-tiles (2 MiB,
  128 fat descriptors each -> full DMA rate) and fold into one
  [128, 4096] accumulator with an in-place DVE chain, hidden under
  the load stream.
- Rows 768..1023 load as four [128, 2, 1024] column-band blocks
  (1 MiB, 4 KiB descriptors). Block c is the LAST data touching its
  1024 columns, so as soon as it lands, those columns fold (two DVE
  adds) and their ones-matmul partition reduce closes on the PE,
  copies to SBUF on ACT, all while later blocks still stream in.

This staggering is what kills the tail: with monolithic row-tiles the
final [128, 4096] reduce is ~9.5us of serial fp32 PE work (LOW_HIGH
double pass) after the last byte; here the closes pipeline with the
load stream and only ~1/4 of them trail it. Host sums the 8 per-core
[1, 4096] partials.
"""

import numpy as np

M_CORES = 8
ROWS, D = 8192, 4096
ROWS_PER_CORE = ROWS // M_CORES  # 1024
P = 128
ROW_TILES = 6  # rows 0..767
BAND_ROWS = 256  # rows 768..1023, loaded as column bands
BAND_J = BAND_ROWS // P  # 2
BAND_W = 1024
N_BANDS = D // BAND_W  # 4
NCHUNK = 512  # fp32 PSUM bank capacity / max fp32 moving free dim

_nc_cache = None


def _build():
    import concourse.tile as tile
    from concourse import bacc, mybir

    nc = bacc.Bacc(None)
    x = nc.declare_dram_parameter(
        "x", [ROWS_PER_CORE, D], mybir.dt.float32, isOutput=False
    )
    out = nc.declare_dram_parameter("out", [1, D], mybir.dt.float32, isOutput=True)

    xband = x[ROW_TILES * P :, :].rearrange("(j p) d -> p j d", p=P)  # [128, 2, 4096]

    with tile.TileContext(nc) as tc:
        with (
            tc.tile_pool(name="xpool", bufs=4) as xpool,
            tc.tile_pool(name="bpool", bufs=3) as bpool,
            tc.tile_pool(name="vpool", bufs=2) as vpool,
            tc.tile_pool(name="singles", bufs=1) as singles,
            tc.tile_pool(name="psum", bufs=4, space="PSUM") as psum_pool,
        ):
            ones = singles.tile([P, 1], mybir.dt.float32)
            nc.vector.memset(ones[:], 1.0)

            osb = singles.tile([1, D], mybir.dt.float32)

            xts = []
            for k in range(ROW_TILES):
                xt = xpool.tile([P, D], mybir.dt.float32, name=f"xt{k}", tag="xt")
                nc.sync.dma_start(xt[:], x[k * P : (k + 1) * P, :])
                xts.append(xt)

            bts = []
            for c in range(N_BANDS):
                bt = bpool.tile([P, BAND_J * BAND_W], mybir.dt.float32,
                                name=f"bt{c}", tag="bt")
                nc.sync.dma_start(
                    bt[:].rearrange("p (j w) -> p j w", j=BAND_J),
                    xband[:, :, c * BAND_W : (c + 1) * BAND_W],
                )
                bts.append(bt)

            # Fold rows 0..767: in-place DVE chain, one add per arrival.
            acc = singles.tile([P, D], mybir.dt.float32)
            nc.vector.tensor_add(acc[:], xts[0][:], xts[1][:])
            for k in range(2, ROW_TILES):
                nc.vector.tensor_add(acc[:], acc[:], xts[k][:])

            # Per column band: fold the two band sub-tiles plus the
            # accumulator slice, close the partition reduce, copy out.
            for c in range(N_BANDS):
                bt = bts[c]
                col = c * BAND_W
                v = vpool.tile([P, BAND_W], mybir.dt.float32, name=f"v{c}", tag="v")
                nc.vector.tensor_add(v[:], bt[:, 0:BAND_W], bt[:, BAND_W : 2 * BAND_W])
                nc.vector.tensor_add(v[:], v[:], acc[:, col : col + BAND_W])
                for s0 in range(0, BAND_W, NCHUNK):
                    ps = psum_pool.tile([1, NCHUNK], mybir.dt.float32,
                                        name=f"ps{c}_{s0}", tag="ps")
                    nc.tensor.matmul(
                        ps[:1], ones[:], v[:, s0 : s0 + NCHUNK],
                        start=True, stop=True,
                    )
                    nc.scalar.copy(osb[:, col + s0 : col + s0 + NCHUNK], ps[:1])

            nc.sync.dma_start(out[:, :], osb[:])

    nc.compile()
    return nc


def _get_nc():
    global _nc_cache
    if _nc_cache is None:
        _nc_cache = _build()
    return _nc_cache


def _run(x_np: np.ndarray, **run_kwargs):
    from concourse.bass_utils import run_bass_kernel_spmd

    nc = _get_nc()
    shards = np.split(x_np, M_CORES, axis=0)
    in_maps = [{"x": np.ascontiguousarray(s)} for s in shards]
    return run_bass_kernel_spmd(nc, in_maps, list(range(M_CORES)), **run_kwargs)


def kernel(x) -> np.ndarray:
    x_np = np.ascontiguousarray(np.asarray(x), dtype=np.float32)
    assert x_np.shape == (ROWS, D), x_np.shape
    res = _run(x_np)
    partials = np.stack([r["out"][0] for r in res.results])
    return partials.sum(axis=0, dtype=np.float32)
